# revision 25
# baseline (speedup 1.0000x reference)
"""Trainium2 Bass kernel for the Burgers PINN problem.

Computes u(x) for IC/BC points and the PDE residual u_t + u*u_x - nu*u_xx
for collocation points, where u is a tanh MLP (2 -> 128 -> ... -> 1, 7
hidden-to-hidden layers).

Strategy:
  - Pure data parallelism: every core gets 1/8 of x_f AND 1/8 of each
    IC/BC set (17408 points per core); MLP weights are replicated.
  - Derivatives via forward-mode Taylor propagation of four streams
    (H, X=+-Hx, Y=+-Ht, Z=Hxx) in transposed layout [features, points]:
        A    = W^T H_prev                       (PE, lhsT = W)
        H    = tanh(A + b)                      (ACT, -> f32r)
        G'   = H^2 - 1 = -tanh'                 (ACT Square + DVE ts)
        X|Y  = G' (.) [AX|AY]   (one broadcast DVE op, sign flips/layer)
        S2   = 2*AX^2 = Square(sqrt2*AX)        (ACT)
        m    = H (.) S2                         (Pool)
        T    = m - AZ                           (DVE)
        Z    = G' (.) T  ( = +Hxx exactly )     (Pool)
  - Matmuls run in float32r (TF32-like, 4x faster than fp32 on the PE;
    measured rel err ~1.5e-4 per matmul, final ~1e-4 vs fp32 reference).
  - Final layer packs [u; +-ux; +-ut; uxx] into one [4, T] PSUM tile via
    four accumulating matmuls with sparse [128,4] lhsT columns.
  - Residual combination + concat on host (tiny); X/Y sign parity after
    7 hidden layers is +1, so no host-side sign fix is needed.
"""

import sys

if "/opt/trn_rl_repo" not in sys.path:
    sys.path.insert(0, "/opt/trn_rl_repo")

import numpy as np

N_CORES = 8
H = 128
L = 7  # hidden-to-hidden layers
NF, N0, NB = 131072, 4096, 2048
NF_C, N0_C, NB_C = NF // N_CORES, N0 // N_CORES, NB // N_CORES
NPTS = N0_C + 2 * NB_C + NF_C  # 17408 points per core
TILE = 512
NTILES = NPTS // TILE  # 34
NU = 0.01 / np.pi

# consts tensor layout (columns of a [128, NCONST] fp32 array)
IBH = 0            # cols 0..6   : b_hid[l]
IB_IN = 7          # col  7      : b_in
IAX = 8            # col  8      : W_in[0, :]        (d a0/dx per partition)
IAT = 9            # col  9      : W_in[1, :]        (d a0/dt per partition)
IAX2 = 10          # col 10      : 2 * W_in[0,:]^2
IWF = 11           # cols 11..26 : four [128,4] lhsT mats, mat m has W_out in col m
NCONST = 27

# how many of the 7 hidden layers compute S = H^2 on Pool instead of ACT
S_ON_POOL_LAYERS = ()

TRACE = False
LAST_RESULTS = None

_CACHE = {}


def _build_bass():
    import concourse.tile as tile
    from concourse import bacc, mybir

    f32 = mybir.dt.float32
    f32r = mybir.dt.float32r
    AF = mybir.ActivationFunctionType
    OP = mybir.AluOpType
    SQRT2 = float(np.sqrt(2.0))

    nc = bacc.Bacc("TRN2", target_bir_lowering=False,
                   detect_race_conditions=False)

    xT = nc.dram_tensor("xt", [2, NPTS], f32, kind="ExternalInput")
    whid = nc.dram_tensor("whid", [L, H, H], f32, kind="ExternalInput")
    win = nc.dram_tensor("win", [2, H], f32, kind="ExternalInput")
    consts = nc.dram_tensor("consts", [H, NCONST], f32, kind="ExternalInput")
    out4 = nc.dram_tensor("out4", [4, NPTS], f32, kind="ExternalOutput")

    with tile.TileContext(nc) as tc:
        with (
            tc.tile_pool(name="wpool", bufs=1) as wp,
            tc.tile_pool(name="spool", bufs=8) as sp,
            tc.tile_pool(name="tpool", bufs=8) as tp,
            tc.tile_pool(name="ppool", bufs=1, space="PSUM") as pp,
        ):
            w_f = wp.tile([H, L * H], f32, tag="whidf")
            for l in range(L):
                nc.sync.dma_start(w_f[:, l * H:(l + 1) * H], whid[l, :, :])
            win_sb = wp.tile([2, H], f32, tag="win")
            nc.sync.dma_start(win_sb[:, :], win[:, :])
            c_sb = wp.tile([H, NCONST], f32, tag="consts")
            nc.sync.dma_start(c_sb[:, :], consts[:, :])


            # one-time rounding of matmul weights to f32r
            w_r = wp.tile([H, L * H], f32r, tag="whidr")
            nc.vector.tensor_copy(w_r[:, :], w_f[:, :])
            wfin_r = wp.tile([H, 16], f32r, tag="wfinr")
            nc.vector.tensor_copy(wfin_r[:, :], c_sb[:, IWF:IWF + 16])

            def col(j):
                return c_sb[:, j:j + 1]

            # tiles 0,1 hold the 1024 IC/BC points: forward pass only.
            AUX_TILES = (N0_C + 2 * NB_C) // TILE  # = 2
            STRIDE = 4  # tile i starts STRIDE layer-slots after tile i-1
            STAGES = L + 2  # input stage, L hidden stages, output stage
            state = {}

            def stage_in(i):
                """Input layer (plain fp32 matmul, K=2) + layer-0 streams."""
                tsl = slice(i * TILE, (i + 1) * TILE)
                r = parity[i]
                aux = i < AUX_TILES
                x_t = sp.tile([2, TILE], f32, tag="xin")
                nc.sync.dma_start(x_t[:, :], xT[:, tsl])
                a = pp.tile([H, TILE], f32, tag=f"pa{r}")
                nc.tensor.matmul(a[:, :], win_sb[:, :], x_t[:, :],
                                 start=True, stop=True)
                yield
                h = sp.tile([H, TILE], f32r, tag="h")
                nc.scalar.activation(h[:, :], a[:, :], AF.Tanh, bias=col(IB_IN))
                xy = z = None
                if not aux:
                    yield
                    s = tp.tile([H, TILE], f32, tag="s")
                    nc.scalar.square(s[:, :], h[:, :])
                    yield
                    g = tp.tile([H, TILE], f32, tag="g")
                    nc.vector.tensor_scalar_sub(g[:, :], s[:, :], 1.0)  # H^2-1
                    yield
                    xy = sp.tile([H, 2, TILE], f32r, tag="xy")
                    nc.vector.tensor_scalar_mul(xy[:, 0, :], g[:, :], col(IAX))
                    yield
                    nc.vector.tensor_scalar_mul(xy[:, 1, :], g[:, :], col(IAT))
                    yield
                    tq = tp.tile([H, TILE], f32, tag="tq")
                    nc.vector.tensor_scalar_mul(tq[:, :], g[:, :], col(IAX2))
                    yield
                    z = sp.tile([H, TILE], f32r, tag="z")
                    nc.gpsimd.tensor_mul(z[:, :], tq[:, :], h[:, :])  # = +Hxx0
                state[i] = (h, xy, z)

            def stage_hidden(i, l):
                r = parity[i]
                aux = i < AUX_TILES
                h, xy, z = state[i]
                wl = w_r[:, l * H:(l + 1) * H]
                a = pp.tile([H, TILE], f32, tag=f"pa{r}")
                nc.tensor.matmul(a[:, :], wl, h[:, :], start=True, stop=True)
                if not aux:
                    axy = pp.tile([H, 2, TILE], f32, tag=f"paxy{r}")
                    az = pp.tile([H, TILE], f32, tag=f"paz{r}")
                    nc.tensor.matmul(axy[:, 0, :], wl, xy[:, 0, :],
                                     start=True, stop=True)
                    yield
                    nc.tensor.matmul(axy[:, 1, :], wl, xy[:, 1, :],
                                     start=True, stop=True)
                    nc.tensor.matmul(az[:, :], wl, z[:, :],
                                     start=True, stop=True)
                yield
                hn = sp.tile([H, TILE], f32r, tag="h")
                nc.scalar.activation(hn[:, :], a[:, :], AF.Tanh,
                                     bias=col(IBH + l))
                xyn = zn = None
                if not aux:
                    yield
                    s2 = tp.tile([H, TILE], f32, tag="s2")
                    nc.scalar.activation(s2[:, :], axy[:, 0, :], AF.Square,
                                         scale=SQRT2)  # = 2*AX^2
                    yield
                    s = tp.tile([H, TILE], f32, tag="s")
                    if l in S_ON_POOL_LAYERS:
                        nc.gpsimd.tensor_mul(s[:, :], hn[:, :], hn[:, :])
                    else:
                        nc.scalar.square(s[:, :], hn[:, :])
                    yield
                    g = tp.tile([H, TILE], f32, tag="g")
                    nc.vector.tensor_scalar_sub(g[:, :], s[:, :], 1.0)
                    yield
                    xyn = sp.tile([H, 2, TILE], f32r, tag="xy")
                    g_b = g[:, :].unsqueeze(1).broadcast_to([H, 2, TILE])
                    nc.vector.tensor_tensor(xyn[:, :, :], axy[:, :, :], g_b,
                                            OP.mult)
                    yield
                    m = tp.tile([H, TILE], f32, tag="m")
                    nc.gpsimd.tensor_mul(m[:, :], hn[:, :], s2[:, :])
                    yield
                    t = tp.tile([H, TILE], f32, tag="t")
                    nc.vector.tensor_sub(t[:, :], m[:, :], az[:, :])
                    yield
                    zn = sp.tile([H, TILE], f32r, tag="z")
                    nc.gpsimd.tensor_mul(zn[:, :], g[:, :], t[:, :])
                state[i] = (hn, xyn, zn)

            def stage_out(i):
                tsl = slice(i * TILE, (i + 1) * TILE)
                r = parity[i]
                aux = i < AUX_TILES
                h, xy, z = state.pop(i)
                o = pp.tile([4, TILE], f32, tag=f"pa{r}")
                if aux:
                    nc.tensor.matmul(o[:, :], wfin_r[:, 0:4], h[:, :],
                                     start=True, stop=True)
                else:
                    rhss = [h[:, :], xy[:, 0, :], xy[:, 1, :], z[:, :]]
                    for mi, rhs in enumerate(rhss):
                        nc.tensor.matmul(o[:, :], wfin_r[:, 4 * mi:4 * (mi + 1)],
                                         rhs, start=(mi == 0), stop=(mi == 3))
                yield
                o_t = sp.tile([4, TILE], f32, tag="ot")
                nc.scalar.copy(o_t[:, :], o[:, :])
                nc.sync.dma_start(out4[:, tsl], o_t[:, :])

            def tile_gen(i):
                """All stages of tile i; yields mark slot boundaries.

                8 slots per tile: input, hidden 0..5, (hidden 6 + output).
                """
                yield from stage_in(i)
                yield "stage"
                for l in range(L):
                    yield from stage_hidden(i, l)
                    if l < L - 1:
                        yield "stage"
                yield from stage_out(i)
                yield "stage"

            # software-pipelined wavefront: tile i starts STRIDE stages after
            # tile i-1; ops of the active tiles are emitted round-robin so the
            # in-order engine queues interleave the two chains.
            # launch order: full tiles first, aux (short) tiles last so the
            # pipeline fills with real work and aux fills the drain.
            order = list(range(AUX_TILES, NTILES)) + list(range(AUX_TILES))
            starts = {}
            t0 = 0
            for k, i in enumerate(order):
                parity[i] = k % 2
                starts[i] = t0
                t0 += STRIDE[k % len(STRIDE)]

            gens = []
            next_k = 0
            slot = 0
            while gens or next_k < NTILES:
                while next_k < NTILES and starts[order[next_k]] <= slot:
                    gens.append(tile_gen(order[next_k]))
                    next_k += 1
                pending = list(gens)
                while pending:
                    for gn in list(pending):
                        tok = next(gn, "done")
                        if tok == "stage":
                            pending.remove(gn)
                        elif tok == "done":
                            pending.remove(gn)
                            gens.remove(gn)
                slot += 1

    nc.compile()
    return nc


def _get_nc():
    if "nc" not in _CACHE:
        _CACHE["nc"] = _build_bass()
    return _CACHE["nc"]


def kernel(x_f, x0_cat, xb_left_cat, xb_right_cat,
           W_in, b_in, W_hid, b_hid, W_out, b_out):
    global LAST_RESULTS
    from concourse.bass_utils import run_bass_kernel_spmd

    f32 = np.float32
    x_f = np.asarray(x_f, f32)
    x0_cat = np.asarray(x0_cat, f32)
    xb_left_cat = np.asarray(xb_left_cat, f32)
    xb_right_cat = np.asarray(xb_right_cat, f32)
    W_in = np.ascontiguousarray(np.asarray(W_in, f32))
    b_in = np.asarray(b_in, f32)
    W_hid = np.ascontiguousarray(np.asarray(W_hid, f32))
    b_hid = np.asarray(b_hid, f32)
    W_out = np.asarray(W_out, f32)
    b_out = np.asarray(b_out, f32)

    consts = np.zeros((H, NCONST), f32)
    consts[:, 0:L] = b_hid.T
    consts[:, IB_IN] = b_in
    consts[:, IAX] = W_in[0]
    consts[:, IAT] = W_in[1]
    consts[:, IAX2] = 2.0 * W_in[0] ** 2
    for mi in range(4):
        consts[:, IWF + 4 * mi + mi] = W_out[:, 0]
    consts = np.ascontiguousarray(consts)

    in_maps = []
    for k in range(N_CORES):
        pts = np.concatenate([
            x0_cat[k * N0_C:(k + 1) * N0_C],
            xb_left_cat[k * NB_C:(k + 1) * NB_C],
            xb_right_cat[k * NB_C:(k + 1) * NB_C],
            x_f[k * NF_C:(k + 1) * NF_C],
        ], axis=0)  # [NPTS, 2]
        in_maps.append({
            "xt": np.ascontiguousarray(pts.T),
            "whid": W_hid,
            "win": W_in,
            "consts": consts,
        })

    nc = _get_nc()
    res = run_bass_kernel_spmd(nc, in_maps, core_ids=list(range(N_CORES)),
                               trace=TRACE)
    LAST_RESULTS = res

    u0_parts, ubl_parts, ubr_parts, r_parts = [], [], [], []
    for k in range(N_CORES):
        o = res.results[k]["out4"]  # [4, NPTS]
        u = o[0] + b_out[0]
        # X/Y parity after 7 hidden layers is +1 (see module docstring)
        ux, ut, uxx = o[1], o[2], o[3]
        u0_parts.append(u[:N0_C])
        ubl_parts.append(u[N0_C:N0_C + NB_C])
        ubr_parts.append(u[N0_C + NB_C:N0_C + 2 * NB_C])
        f = slice(N0_C + 2 * NB_C, None)
        r_parts.append(ut[f] + u[f] * ux[f] - NU * uxx[f])

    out = np.concatenate(u0_parts + ubl_parts + ubr_parts + r_parts)
    return np.ascontiguousarray(out.reshape(-1, 1).astype(f32))
